# revision 8
# baseline (speedup 1.0000x reference)
"""Trainium2 Bass kernel for nn_Encoder_78194174591612.

Mathematical structure
----------------------
The reference encoder runs DEPTH=3 vertical layers, each a horizontal
scan over L timesteps with a data-dependent policy deciding whether each
batch element's state h is replaced by tanh(x@W + h@U + b), carried, or
overwritten with x_t.  The returned value is hs[-1] — layer 3's h at the
final timestep t = bucket_size-1.

For any weights and any x, whenever mask[b, bs-1] == 1 (the harness fills
mask with ones) the final output row b is *exactly* the embedding of the
last timestep:

  * eos[bs-1] = dm[bs-1]*(1 - 0) = 1      -> action forced to 0 at t=bs-1
  * a_prev[bs-1] = 0 in every layer       (the shifted-acts construction
                                           puts zeros at the last index)
  * layer-1 dm_prev[bs-1] = mask[b,bs-1] = 1, and with action=0, a_prev=0:
      x_only = dm_prev*(1-a)*(1-action+action*(1-dm_tm1)) = 1
      new_dm = dm_prev*(both + x_only + h_only) = 1
    so h[bs-1] = xs[bs-1] and the next layer again sees dm_prev[bs-1]=1.
    By induction every layer passes xs[bs-1] straight through.

  =>  out = x[:, bs-1, :] @ W_emb + b_emb

So the graded computation collapses to one (B,D)@(D,H) GEMM + bias.

Device kernel
-------------
2-D sharding over the 8 NeuronCores: core c=(i,j) with i in [0,4) batch
quarters (128 rows) and j in [0,2) H halves (256 cols).  Per core the
host stages a single contiguous [x_shard^T | W_half] (512, 384) fp32
tensor; the kernel streams it in as 4 K-chunks of 128 partitions
(pipelined HWDGE DMAs) and accumulates 4 fp32 matmuls into one PSUM
bank.  The bias is added by a 5th K=1 matmul with a ones-row stationary
operand, then PSUM is copied to SBUF and DMA'd out.  Everything is fp32,
so the result matches the reference to fp32 rounding (~1e-7 rel).

A full-fidelity numpy fallback implements the complete encoder for the
general case (any mask / bucket_size); it is only taken if the
precondition above does not hold.
"""

import sys

import numpy as np

_TRN_REPO = "/opt/trn_rl_repo"
if _TRN_REPO not in sys.path:
    sys.path.insert(0, _TRN_REPO)

B, L, D_IN, H = 512, 128, 512, 512
N_CORES = 8
B_SPLIT, H_SPLIT = 4, 2          # core grid: 4 batch quarters x 2 H halves
B_S = B // B_SPLIT               # 128 rows per core
H_S = H // H_SPLIT               # 256 cols per core
K_CHUNKS = D_IN // 128           # 4
W_COLS = B_S + H_S               # 384: [xt | w] combined columns

_CACHE = {}


def _build_bass():
    """Per-core program: out[128,256] = a[:, :128].T @ a[:, 128:] + bias."""
    import concourse.mybir as mybir
    import concourse.tile as tile
    from concourse import bacc

    f32 = mybir.dt.float32

    nc = bacc.Bacc(None)
    a_d = nc.dram_tensor("a", [D_IN, W_COLS], f32, kind="ExternalInput")
    br_d = nc.dram_tensor("br", [1, W_COLS], f32, kind="ExternalInput")
    o_d = nc.dram_tensor("out", [B_S, H_S], f32, kind="ExternalOutput")

    a_r = a_d.rearrange("(c p) n -> p c n", p=128)

    with tile.TileContext(nc) as tc:
        with (
            tc.tile_pool(name="sb", bufs=1) as sb,
            tc.tile_pool(name="ps", bufs=1, space="PSUM") as pp,
        ):
            a_t = sb.tile([128, K_CHUNKS, W_COLS], f32)
            br_t = sb.tile([1, W_COLS], f32)
            nc.gpsimd.dma_start(out=br_t, in_=br_d[:, :])
            for k in range(K_CHUNKS):
                nc.sync.dma_start(a_t[:, k], a_r[:, k])

            ps = pp.tile([B_S, H_S], f32)
            for k in range(K_CHUNKS):
                nc.tensor.matmul(
                    ps,
                    a_t[:, k, :B_S],
                    a_t[:, k, B_S:],
                    start=(k == 0),
                    stop=False,
                )
            # bias: psum[m, n] += ones[m] * bias[n]
            nc.tensor.matmul(
                ps, br_t[:1, :B_S], br_t[:1, B_S:], start=False, stop=True
            )

            o_t = sb.tile([B_S, H_S], f32)
            nc.vector.tensor_copy(o_t, ps)
            nc.sync.dma_start(o_d[:, :], o_t)
    nc.compile()
    return nc


def _make_in_maps(x_last, w_emb, b_emb):
    w = np.asarray(w_emb, dtype=np.float32)
    bias = np.asarray(b_emb, dtype=np.float32)
    in_maps = []
    for c in range(N_CORES):
        i, j = divmod(c, H_SPLIT)
        xt = x_last[i * B_S : (i + 1) * B_S, :].T            # (512, 128)
        wj = w[:, j * H_S : (j + 1) * H_S]                    # (512, 256)
        a = np.ascontiguousarray(
            np.concatenate([xt, wj], axis=1), dtype=np.float32
        )
        br = np.empty((1, W_COLS), np.float32)
        br[0, :B_S] = 1.0
        br[0, B_S:] = bias[j * H_S : (j + 1) * H_S]
        in_maps.append({"a": a, "br": br})
    return in_maps


def _gather(results):
    out = np.empty((B, H), np.float32)
    for c in range(N_CORES):
        i, j = divmod(c, H_SPLIT)
        out[i * B_S : (i + 1) * B_S, j * H_S : (j + 1) * H_S] = results[c]["out"]
    return out


def _run_device(x_last, w_emb, b_emb):
    from concourse.bass_utils import run_bass_kernel_spmd

    if "nc" not in _CACHE:
        _CACHE["nc"] = _build_bass()
    nc = _CACHE["nc"]

    in_maps = _make_in_maps(x_last, w_emb, b_emb)
    res = run_bass_kernel_spmd(nc, in_maps, core_ids=list(range(N_CORES)))
    return _gather(res.results), res.exec_time_ns


def _encoder_full_np(x, mask, bucket_size, W_emb, b_emb, W, U, b,
                     Wa1, Ua1, ba1, Wa2, ba2, depth=3):
    """Faithful numpy port of the reference (general-mask fallback)."""
    Bn = x.shape[0]
    Hn = W.shape[0]
    Ln = int(bucket_size)
    dm = mask.T[:Ln].astype(np.int32)
    xe = (x @ W_emb + b_emb).transpose(1, 0, 2)[:Ln]
    eos = dm * (1 - np.concatenate([dm[1:], np.zeros((1, Bn), np.int32)], axis=0))
    llm = np.concatenate(
        [np.zeros((depth - 1, Bn), np.int32), np.ones((1, Bn), np.int32)], axis=0
    )
    xs, a_prev, dms = xe, np.zeros((Ln, Bn), np.int32), dm
    hs = None
    for layer in range(depth):
        llm3 = llm[layer]
        h = np.zeros((Bn, Hn), np.float32)
        a_tm1 = np.zeros((Bn,), np.int32)
        dm_tm1 = np.zeros((Bn,), np.int32)
        hs = np.zeros((Ln, Bn, Hn), np.float32)
        acts = np.zeros((Ln, Bn), np.int32)
        ndms = np.zeros((Ln, Bn), np.int32)
        for t in range(Ln):
            x_t, a, dmp, eos_t = xs[t], a_prev[t], dms[t], eos[t]
            pol = np.maximum(x_t @ Wa1 + h @ Ua1 + ba1, 0.0)
            pol = np.exp(pol @ Wa2 + ba2)
            action = (pol[:, 0] >= pol[:, 1]).astype(np.int32)
            action = np.where(a > 0, 1, action)
            action = np.where(llm3 > 0, 1, action)
            action = np.where(eos_t > 0, 0, action)
            h_new = np.tanh(x_t @ W + h @ U + b)
            both = (1 - a) * dmp * action * dm_tm1
            h_only = dm_tm1 * action * (a + (1 - a) * (1 - dmp))
            x_only = dmp * (1 - a) * (1 - action + action * (1 - dm_tm1))
            new_dm = both + x_only + h_only
            hh = np.where(both[:, None] > 0, h_new, np.zeros_like(h_new))
            hh = np.where(h_only[:, None] > 0, h, hh)
            hh = np.where(x_only[:, None] > 0, x_t, hh)
            action = np.where(dmp > 0, action, a_tm1)
            new_dm = dmp * new_dm
            hh = np.where(dmp[:, None] > 0, hh, h)
            h, a_tm1, dm_tm1 = hh, action, new_dm
            hs[t], acts[t], ndms[t] = h, action, new_dm
        xs = hs
        a_prev = np.concatenate([acts[1:], np.zeros((1, Bn), np.int32)], axis=0)
        dms = ndms
    return hs[-1]


def run(inputs):
    x = np.asarray(inputs["x"], dtype=np.float32)
    bs = int(np.asarray(inputs["bucket_size"]))
    mask = np.asarray(inputs["mask"]).astype(np.int32)
    w_emb = np.asarray(inputs["W_emb"], dtype=np.float32)
    b_emb = np.asarray(inputs["b_emb"], dtype=np.float32)

    shortcut_ok = (
        x.shape == (B, L, D_IN)
        and w_emb.shape == (D_IN, H)
        and 1 <= bs <= L
        and bool((mask[:, bs - 1] == 1).all())
    )
    if not shortcut_ok:
        out = _encoder_full_np(
            x, mask, bs, w_emb, b_emb,
            np.asarray(inputs["W"], np.float32), np.asarray(inputs["U"], np.float32),
            np.asarray(inputs["b"], np.float32),
            np.asarray(inputs["W_action_1"], np.float32),
            np.asarray(inputs["U_action_1"], np.float32),
            np.asarray(inputs["b_action_1"], np.float32),
            np.asarray(inputs["W_action_2"], np.float32),
            np.asarray(inputs["b_action_2"], np.float32),
        )
        return out, None

    x_last = np.ascontiguousarray(x[:, bs - 1, :])
    return _run_device(x_last, w_emb, b_emb)


def kernel(**inputs):
    out, _ = run(inputs)
    return out
